# revision 33
# baseline (speedup 1.0000x reference)
"""Causal attention head (B=8, C=2048, E=1024, H=64) with post-softmax query-row
zero mask, on 8 TRN2 NeuronCores — data-parallel over batch (one batch per core).

Sparse trick: ~50% of query rows are zero-masked post-softmax, so their outputs
are never needed. The host gathers the kept query positions (sorted), pads them
at the FRONT to a fixed QK_PAD, and the device computes attention only for
gathered query columns. Causality for gathered columns is enforced by
(a) restricting each score tile's moving range to columns whose position can
reach that key chunk (host-baked, min over cores) and (b) one narrow host-built
0/1 mask multiply per boundary tile (per-core data).

Pipeline layout: DMA triggers are ordered so consumers unblock in program
order (xt0 halves, xq[wq|q0], xq1, masks+xq2, xt1, xt2, xt3 halves);
attention j-segments are emitted between kv-projection chunks as their k/q
inputs land, so the PE queue never blocks on a far-future dependency and the
Scalar engine's exp stream (the attention-phase bottleneck, ~(N+352)/1.2 ns
per instruction) starts as early as the data allows and rarely drains. AV
matmuls trail their scores by two pair-steps so the exp round-trip never
paces the PE queue; boundary-mask multiplies ride on GpSimd inside that
slack. The last output chunk ships in two pieces: columns finalized by j11
leave while the j12-15 tail still computes.

Per-core dataflow (matmuls bf16/fp8 -> f32 PSUM):
  [Wk|Wv] packed projection over all 2048 key positions -> k_sb, vt_sb
        (single [64,512] Vector casts; k/q stay in partitions 0:64 and all
        scores run on PE rows 0:64 — a serial-score layout measured faster
        than row-tiled pairs because every k-chunk handoff is one cast).
  v^T PE-transposed into v_aug tiles [128j, 65] with column 64 = 1.0 (the AV
        matmul then also emits softmax denominators as row 64).
  [Wq|Wq] projection over gathered x columns (fp8; quantization error
        attenuated ~45x by the C**-0.5 score scale) -> q_sb.
  scoresT[j, q] = k-chunk (stationary) @ q (moving); exp on ScalarE with the
        C**-0.5 scale fused, two j-tiles per exp op when the garbage span is
        under 352 columns; AV accumulates outT[65, q] over j-chunks.
  The host divides by the denominators and scatters columns back to rows
  (masked rows stay zero) while unsharding. Output ships bf16 (unnormalized
  sums; the host divide restores precision headroom).
"""

import numpy as np
import ml_dtypes

import concourse.bass as bass
import concourse.bacc as bacc
import concourse.mybir as mybir
import concourse.tile as tile
from concourse.bass_utils import run_bass_kernel_spmd
from concourse.masks import make_identity

B, C, E, H = 8, 2048, 1024, 64
EC = E // 128          # 8 contraction chunks
KC = C // 512          # 4 key/value column chunks of 512
NJ = C // 128          # 16 key chunks of 128
QK_PAD = 1536          # gathered queries padded (front) to this
QKC = QK_PAD // 512    # 3 gathered-query chunks
SCALE = float(C) ** -0.5
BF16 = mybir.dt.bfloat16
F32 = mybir.dt.float32

_CACHED = {}


def _plan(zero_mask):
    """Host-side plan: per-core gathered positions + shared baked bounds."""
    zm = np.asarray(zero_mask)
    pos = []   # per core: [QK_PAD] int, -1 for front pads
    for b in range(B):
        kept = np.nonzero(~zm[b])[0]
        assert len(kept) <= QK_PAD, len(kept)
        p = np.full(QK_PAD, -1, dtype=np.int64)
        p[QK_PAD - len(kept):] = kept
        pos.append(p)
    pos = np.stack(pos)  # [B, QK_PAD]

    # qoff[b, ck, jc] = #cols in chunk ck with pos < 128*jc (cols are sorted)
    qoff = np.zeros((B, QKC, NJ + 1), dtype=np.int64)
    for ck in range(QKC):
        pc = pos[:, ck * 512:(ck + 1) * 512]
        for jc in range(NJ + 1):
            qoff[:, ck, jc] = (pc < 128 * jc).sum(axis=1)
    jmax = []   # per chunk: number of key chunks any core needs
    mm_off = []  # baked matmul start col (min over cores)
    mk_end = []  # baked mask end col (max over cores)
    for ck in range(QKC):
        jm = 0
        for jc in range(NJ):
            if (qoff[:, ck, jc] < 512).any():
                jm = jc + 1
        jmax.append(jm)
        mm_off.append([int(qoff[:, ck, jc].min()) for jc in range(NJ)])
        mk_end.append([int(qoff[:, ck, jc + 1].max()) for jc in range(NJ)])
    return pos, qoff, tuple(jmax), mm_off, mk_end


def _build(jmax, mm_off, mk_end, mask_w):
    nc = bacc.Bacc("TRN2", target_bir_lowering=False, debug=False, num_devices=B)
    FP8 = mybir.dt.float8e4
    MW = max(mask_w, 1)
    W0F = mm_off[0][0] & ~3       # pad columns trimmed from xq chunk 0
    QW = [512 - W0F, 512, 512]    # shipped query-chunk widths
    XOFF = EC * 128   # wkv columns at the head of xt
    QOFF = EC * 128   # [wq|wq] columns at the head of xq (duplicated: one
                      # matmul then emits q into both PSUM partition halves)
    # xq layout: [wq | xq0 (trimmed) | xq1 | masks | xq2]
    QB = [QOFF]                     # per-chunk column base inside xq
    QB.append(QOFF + EC * QW[0])
    MSK0 = QB[1] + EC * 512         # masks sit between xq1 and xq2
    QB.append(MSK0 + MW)
    XQW = QB[2] + EC * 512
    xt_ext = nc.dram_tensor("xt", [128, XOFF + KC * EC * 512], BF16, kind="ExternalInput")
    xq_ext = nc.dram_tensor("xq", [128, XQW], FP8, kind="ExternalInput")
    out_ext = nc.dram_tensor("out", [H + 1, QK_PAD], BF16, kind="ExternalOutput")

    with tile.TileContext(nc) as tc:
        with (
            tc.tile_pool(name="const", bufs=1) as const_pool,
            tc.tile_pool(name="acts", bufs=1) as act_pool,
            tc.tile_pool(name="p", bufs=4) as p_pool,
            tc.tile_pool(name="osb", bufs=2) as o_pool,
            tc.tile_pool(name="mmp", bufs=2, space="PSUM") as mmp_pool,
            tc.tile_pool(name="mms", bufs=2, space="PSUM") as mms_pool,
            tc.tile_pool(name="po", bufs=2, space="PSUM") as po_pool,
        ):
            ident = const_pool.tile([128, 128], BF16)
            xt_all = act_pool.tile([128, XOFF + KC * EC * 512], BF16)
            xq_all = act_pool.tile([128, XQW], FP8)
            msk_sb = xq_all[:, MSK0:MSK0 + MW]
            wkv_sb = xt_all[:, 0:XOFF]
            xt_sb = xt_all[:, XOFF:]
            wq_sb = xq_all[:, 0:QOFF]

            # ---- input DMA triggers, in consumer order (sync queue) ----
            half = EC * 512 // 2
            quart = EC * 512 // 4

            def xt_rng(lo, hi):
                nc.sync.dma_start(xt_all[:, lo:hi], xt_ext.ap()[:, lo:hi])

            def xq_rng(lo, hi):
                nc.sync.dma_start(xq_all[:, lo:hi], xq_ext.ap()[:, lo:hi])

            xt_rng(0, XOFF + half)                        # wkv + xt0 e0-3
            xt_rng(XOFF + half, XOFF + EC * 512)          # xt0 e4-7
            xq_rng(0, QB[1])                              # wq + xq0
            xq_rng(QB[1], MSK0)                           # xq1
            xq_rng(MSK0, XQW)                             # masks + xq2
            xt_rng(XOFF + EC * 512, XOFF + 2 * EC * 512)  # xt1
            xt_rng(XOFF + 2 * EC * 512, XOFF + 3 * EC * 512)  # xt2
            xt_rng(XOFF + 3 * EC * 512, XOFF + 3 * EC * 512 + half)  # xt3 e0-3
            xt_rng(XOFF + 3 * EC * 512 + half, XOFF + 4 * EC * 512)  # xt3 e4-7

            make_identity(nc, ident[:])
            # touch Exp once so the ACT table set loads during the DMA phase
            warm = const_pool.tile([1, 1], F32)
            nc.scalar.activation(warm[:], ident[0:1, 0:1],
                                 mybir.ActivationFunctionType.Exp)

            # k and q live in partitions 0:64 only; scores always run on PE
            # array rows 0:64 (serial on the array, but every k-chunk handoff
            # is a single Vector cast — no cross-engine duplication chain,
            # which measured faster than row-tiled score pairs)
            k_sb = act_pool.tile([64, C], BF16)
            vt_sb = act_pool.tile([64, C], BF16)
            q_sb = act_pool.tile([64, QK_PAD], BF16)
            vaug_sb = act_pool.tile([128, NJ * (H + 1)], BF16)
            nc.vector.memset(vaug_sb[:], 1.0)

            I32 = mybir.dt.int32

            def kv_mm(c, pq, e):
                nc.tensor.matmul(
                    pq[:], wkv_sb[:, e * 128:(e + 1) * 128],
                    xt_sb[:, (c * EC + e) * 512:(c * EC + e + 1) * 512],
                    start=(e == 0), stop=(e == EC - 1), skip_group_check=True)

            def kv_casts(c, pq):
                csl = slice(c * 512, (c + 1) * 512)
                nc.vector.tensor_copy(k_sb[:, csl], pq[0:64, :])
                nc.vector.tensor_copy(vt_sb[:, csl], pq[64:128, :])

            def kv_group(c):
                pq = mmp_pool.tile([128, 512], F32, tag="mm", name=f"pq{c}")
                for e in range(EC):
                    kv_mm(c, pq, e)
                kv_casts(c, pq)

            def trp(c):
                for jj in range(4):
                    jc = 4 * c + jj
                    pt = mmp_pool.tile([128, H], BF16, tag="mm", name=f"pt{jc}")
                    nc.tensor.transpose(
                        pt[:], vt_sb[:, jc * 128:(jc + 1) * 128],
                        ident[0:64, 0:64])
                    nc.vector.tensor_copy(
                        vaug_sb[:, jc * (H + 1): jc * (H + 1) + H], pt[:])

            def q_proj(ck):
                # [wq|wq] stationary: one matmul per e-slice emits q into both
                # PSUM partition halves at once; a single [128, w] cast lands
                # q duplicated in both SBUF halves.
                w = QW[ck]
                base = QB[ck]
                pv = mmp_pool.tile([128, 512], F32, tag="mm", name=f"pv{ck}")
                for e in range(EC):
                    q_mm(ck, pv, e)
                q_cast(ck, pv)

            def q_mm(ck, pv, e):
                w = QW[ck]
                base = QB[ck]
                nc.tensor.matmul(
                    pv[:, 0:w], wq_sb[:, e * 128:(e + 1) * 128],
                    xq_all[:, base + e * w:base + (e + 1) * w],
                    start=(e == 0), stop=(e == EC - 1), skip_group_check=True)

            def q_cast(ck, pv):
                w = QW[ck]
                lo = ck * 512 + (512 - w)
                nc.vector.tensor_copy(q_sb[:, lo:(ck + 1) * 512], pv[0:64, 0:w])

            # ---- attention machinery ----
            # mask tile packing offsets (shared layout; content is per-core)
            mask_offs = {}
            off = 0
            for ck in range(QKC):
                for jc in range(jmax[ck]):
                    qo, me = mm_off[ck][jc], mk_end[ck][jc]
                    if me > qo and qo < 512:
                        mask_offs[(ck, jc)] = off
                        off += me - qo

            tiles = {ck: [(jc, mm_off[ck][jc], mk_end[ck][jc])
                          for jc in range(jmax[ck]) if mm_off[ck][jc] < 512]
                     for ck in range(QKC)}
            po_state = {}  # ck -> [po_tile, n_av_done, n_av_total]
            pend = []      # delayed AV work: (ck, pair, p_t)

            def flush_av():
                # AVs run one pair-step late so the PE queue never waits on
                # the exp round-trip; the mask multiply (GpSimd) hides in the
                # same slack.
                while pend:
                    ck, pair, p_t = pend.pop(0)
                    st = po_state[ck]
                    for h, (jc, qo, me) in enumerate(pair):
                        nc.tensor.matmul(
                            st[0][:, qo:512],
                            vaug_sb[:, jc * (H + 1):(jc + 1) * (H + 1)],
                            p_t[:, h * 512 + qo:(h + 1) * 512],
                            start=(st[1] == 0), stop=(st[1] == st[2] - 1))
                        st[1] += 1

            def att_step(ck, pair):
                if ck not in po_state:
                    po_state[ck] = [po_pool.tile([H + 1, 512], F32, tag="po",
                                                 name=f"po{ck}"),
                                    0, len(tiles[ck])]
                ps = mms_pool.tile([128, 1024], F32, tag="mms", name="ps")
                p_t = p_pool.tile([128, 1024], BF16, tag="p", name="p_t")
                for h, (jc, qo, me) in enumerate(pair):
                    nc.tensor.matmul(
                        ps[:, h * 512 + qo:(h + 1) * 512],
                        k_sb[:, jc * 128:(jc + 1) * 128],
                        q_sb[:, ck * 512 + qo:(ck + 1) * 512],
                        start=True, stop=True, skip_group_check=True)
                if len(pair) == 2 and pair[1][1] < 352:
                    lo = pair[0][1]
                    nc.scalar.activation(
                        p_t[:, lo:1024], ps[:, lo:1024],
                        mybir.ActivationFunctionType.Exp, scale=SCALE)
                else:
                    for h, (jc, qo, me) in enumerate(pair):
                        nc.scalar.activation(
                            p_t[:, h * 512 + qo:(h + 1) * 512],
                            ps[:, h * 512 + qo:(h + 1) * 512],
                            mybir.ActivationFunctionType.Exp, scale=SCALE)
                for h, (jc, qo, me) in enumerate(pair):
                    if me > qo:  # boundary mask multiply (host-built content)
                        mo = mask_offs[(ck, jc)]
                        nc.gpsimd.tensor_mul(
                            p_t[:, h * 512 + qo:h * 512 + me],
                            p_t[:, h * 512 + qo:h * 512 + me],
                            msk_sb[:, mo:mo + (me - qo)])
                return (ck, pair, p_t)

            def att(ck, j_lo, j_hi):
                seg = [t for t in tiles[ck] if j_lo <= t[0] <= j_hi]
                i = 0
                while i < len(seg):
                    pair = seg[i:i + 2]
                    item = att_step(ck, pair)
                    # keep TWO steps in flight: scores(i) then AV(i-2), so the
                    # exp(i-1) -> AV(i-1) -> scores(i+1) chain never paces the
                    # PE queue (PSUM slot rotation imposes i-2 anyway)
                    while len(pend) > 1:
                        ck_o, pair_o, p_o = pend.pop(0)
                        st = po_state[ck_o]
                        for h, (jc, qo, me) in enumerate(pair_o):
                            nc.tensor.matmul(
                                st[0][:, qo:512],
                                vaug_sb[:, jc * (H + 1):(jc + 1) * (H + 1)],
                                p_o[:, h * 512 + qo:(h + 1) * 512],
                                start=(st[1] == 0), stop=(st[1] == st[2] - 1))
                            st[1] += 1
                    pend.append(item)
                    i += len(pair)

            def close_part(ck, lo, hi, last):
                # ship unnormalized outT + sums row; the host divides while
                # unsharding (removes the recip chain from the critical tail)
                o_t = o_pool.tile([H + 1, 512], BF16, tag="o", name=f"o{ck}_{lo}")
                if last:
                    nc.scalar.copy(o_t[:, lo:hi], po_state[ck][0][:, lo:hi])
                else:
                    nc.vector.tensor_copy(o_t[:, lo:hi], po_state[ck][0][:, lo:hi])
                nc.sync.dma_start(
                    out_ext.ap()[:, ck * 512 + lo:ck * 512 + hi], o_t[:, lo:hi])

            def att_close(ck, lo=None):
                flush_av()
                st = po_state[ck]
                assert st[1] == st[2], (ck, st[1], st[2])
                close_part(ck, lo if lo is not None else mm_off[ck][0], 512,
                           last=(ck == 2))

            # ---- schedule: consumers emitted as their inputs land, AVs one
            # step late, kv/q/transpose work woven between attention steps ----
            # fused kv0 + q1: q1's matmuls weave into kv0's tail (their xq
            # DMA lands later than xt0's) so q1's cast chases k0's and the
            # exp stream starts as early as the data allows
            pq0 = mmp_pool.tile([128, 512], F32, tag="mm", name="pq0")
            pv1 = mmp_pool.tile([128, 512], F32, tag="mm", name="pv1")
            for e in range(EC):
                kv_mm(0, pq0, e)
                if e >= 4:
                    q_mm(1, pv1, e - 4)
            kv_casts(0, pq0)
            for e in range(4, EC):
                q_mm(1, pv1, e)
            q_cast(1, pv1)
            q_proj(2)
            q_proj(0)
            trp(0)
            att(0, 0, 0)
            att_close(0)
            att(1, 0, 3)
            att(2, 0, 3)
            kv_group(1)
            trp(1)
            # kv2/kv3 hoisted ahead of the exp-bound attention segments: the
            # PE has slack there, so k2/k3 casts land before the exp stream
            # needs their scores (their old positions stalled it ~5us)
            kv_group(2)
            att(1, 4, 7)
            att(2, 4, 7)
            trp(2)
            kv_group(3)
            trp(3)
            att(1, 8, 8)
            att_close(1)
            att(2, 8, 11)
            flush_av()
            # columns below the j12 start are final after j11's AV: ship them
            # while the j12-15 scores/exps still run
            qo12 = mm_off[2][12]
            close_part(2, 0, qo12, last=False)
            att(2, 12, 15)
            att_close(2, lo=qo12)

    nc.compile()
    return nc


def _pack_masks(pos, jmax, mm_off, mk_end):
    """Per-core packed boundary masks: msk[j_local, off+q-qo] = (128jc + j_local <= pos[q])."""
    total = 0
    spans = []
    for ck in range(len(jmax)):
        for jc in range(jmax[ck]):
            qo, me = mm_off[ck][jc], mk_end[ck][jc]
            if me > qo and qo < 512:
                spans.append((ck, jc, qo, me, total))
                total += me - qo
    bf = ml_dtypes.bfloat16
    masks = np.zeros((B, 128, max(total, 1)), dtype=np.float32)
    jl = np.arange(128)[:, None]
    for b in range(B):
        for ck, jc, qo, me, off in spans:
            pq = pos[b, ck * 512 + qo: ck * 512 + me][None, :]
            masks[b, :, off:off + (me - qo)] = (128 * jc + jl <= pq)
    return masks.astype(bf), total


def _sbufify(w):  # [E, M] -> [128, EC*M]: w_t[p, e*M+m] = w[e*128+p, m]
    M = w.shape[1]
    return np.ascontiguousarray(
        w.reshape(EC, 128, M).transpose(1, 0, 2).reshape(128, EC * M))


def _retile_cols(xt, ncols, w=512):  # [E, ncols] -> [128, (ncols/w)*EC*w] chunk-major
    return np.ascontiguousarray(
        xt.reshape(EC, 128, ncols // w, w).transpose(1, 2, 0, 3)
        .reshape(128, (ncols // w) * EC * w))


def make_in_maps(x, Wq, Wk, Wv, zero_mask):
    x = np.asarray(x)
    pos, qoff, jmax, mm_off, mk_end = _plan(zero_mask)
    masks, mask_w = _pack_masks(pos, jmax, mm_off, mk_end)
    bf = ml_dtypes.bfloat16
    f8 = ml_dtypes.float8_e4m3fn
    W0F = mm_off[0][0] & ~3
    wkv = _sbufify(np.concatenate([np.asarray(Wk), np.asarray(Wv)], 1)).astype(bf)
    wq = _sbufify(np.concatenate([np.asarray(Wq), np.asarray(Wq)], 1)).astype(f8)
    maps = []
    for b in range(B):
        xtb = np.ascontiguousarray(x[b].T.astype(np.float32))
        xqb = np.zeros((E, QK_PAD), dtype=np.float32)
        real = pos[b] >= 0
        xqb[:, real] = xtb[:, pos[b][real]]
        # chunk 0 ships trimmed (cols W0F:512 only; the rest are pads on
        # every core), chunks 1-2 full width
        xq0 = np.ascontiguousarray(
            xqb[:, W0F:512].reshape(EC, 128, 512 - W0F)
            .transpose(1, 0, 2).reshape(128, EC * (512 - W0F))).astype(f8)
        xq1 = _retile_cols(xqb[:, 512:1024], 512).astype(f8)
        xq2 = _retile_cols(xqb[:, 1024:1536], 512).astype(f8)
        xq_packed = np.concatenate(  # [wq | xq0 | xq1 | masks | xq2]
            [wq, xq0, xq1, masks[b].astype(f8), xq2], axis=1)
        xt_packed = np.concatenate([wkv, _retile_cols(xtb, C).astype(bf)], axis=1)
        maps.append({
            "xt": np.ascontiguousarray(xt_packed),
            "xq": np.ascontiguousarray(xq_packed),
        })
    return maps, (pos, jmax, mm_off, mk_end, mask_w)


def kernel(x, Wq, Wk, Wv, zero_mask):
    in_maps, (pos, jmax, mm_off, mk_end, mask_w) = make_in_maps(
        x, Wq, Wk, Wv, zero_mask)
    key = (jmax, tuple(map(tuple, mm_off)), tuple(map(tuple, mk_end)), mask_w)
    if _CACHED.get("key") != key:
        _CACHED["nc"] = _build(jmax, mm_off, mk_end, mask_w)
        _CACHED["key"] = key
    res = run_bass_kernel_spmd(_CACHED["nc"], in_maps, core_ids=list(range(B)))
    out = np.zeros((B, C, H), dtype=np.float32)
    for b in range(B):
        r = res.results[b]["out"].astype(np.float32)  # [H+1, QK_PAD]
        real = pos[b] >= 0
        out[b][pos[b][real]] = (r[:H, real] / r[H:H + 1, real]).T
    return out


# revision 34
# speedup vs baseline: 1.1885x; 1.1885x over previous
"""Causal attention head (B=8, C=2048, E=1024, H=64) with post-softmax query-row
zero mask, on 8 TRN2 NeuronCores — data-parallel over batch (one batch per core).

Sparse trick: ~50% of query rows are zero-masked post-softmax, so their outputs
are never needed. The host gathers the kept query positions (sorted), pads them
at the FRONT to a fixed QK_PAD, and the device computes attention only for
gathered query columns. Causality for gathered columns is enforced by
(a) restricting each score tile's moving range to columns whose position can
reach that key chunk (host-baked, min over cores) and (b) one narrow host-built
0/1 mask multiply per boundary tile (per-core data).

Pipeline layout: DMA triggers are ordered so consumers unblock in program
order (xt0 halves, xq[wq|q0], xq1, masks+xq2, xt1, xt2, xt3 halves);
attention j-segments are emitted between kv-projection chunks as their k/q
inputs land, so the PE queue never blocks on a far-future dependency and the
Scalar engine's exp stream (the attention-phase bottleneck, ~(N+352)/1.2 ns
per instruction) starts as early as the data allows and rarely drains. AV
matmuls trail their scores by two pair-steps so the exp round-trip never
paces the PE queue; boundary-mask multiplies ride on GpSimd inside that
slack. The last output chunk ships in two pieces: columns finalized by j11
leave while the j12-15 tail still computes.

Per-core dataflow (matmuls bf16/fp8 -> f32 PSUM):
  [Wk|Wv] packed projection over all 2048 key positions -> k_sb, vt_sb
        (single [64,512] Vector casts; k/q stay in partitions 0:64 and all
        scores run on PE rows 0:64 — a serial-score layout measured faster
        than row-tiled pairs because every k-chunk handoff is one cast).
  v^T PE-transposed into v_aug tiles [128j, 65] with column 64 = 1.0 (the AV
        matmul then also emits softmax denominators as row 64).
  [Wq|Wq] projection over gathered x columns (fp8; quantization error
        attenuated ~45x by the C**-0.5 score scale) -> q_sb.
  scoresT[j, q] = k-chunk (stationary) @ q (moving); exp on ScalarE with the
        C**-0.5 scale fused, two j-tiles per exp op when the garbage span is
        under 352 columns; AV accumulates outT[65, q] over j-chunks.
  The host divides by the denominators and scatters columns back to rows
  (masked rows stay zero) while unsharding. Output ships bf16 (unnormalized
  sums; the host divide restores precision headroom).
"""

import numpy as np
import ml_dtypes

import concourse.bass as bass
import concourse.bacc as bacc
import concourse.mybir as mybir
import concourse.tile as tile
from concourse.bass_utils import run_bass_kernel_spmd
from concourse.masks import make_identity

B, C, E, H = 8, 2048, 1024, 64
EC = E // 128          # 8 contraction chunks
KC = C // 512          # 4 key/value column chunks of 512
NJ = C // 128          # 16 key chunks of 128
QK_PAD = 1536          # gathered queries padded (front) to this
QKC = QK_PAD // 512    # 3 gathered-query chunks
SCALE = float(C) ** -0.5
BF16 = mybir.dt.bfloat16
F32 = mybir.dt.float32

_CACHED = {}


def _plan(zero_mask):
    """Host-side plan: per-core gathered positions + shared baked bounds."""
    zm = np.asarray(zero_mask)
    pos = []   # per core: [QK_PAD] int, -1 for front pads
    for b in range(B):
        kept = np.nonzero(~zm[b])[0]
        assert len(kept) <= QK_PAD, len(kept)
        p = np.full(QK_PAD, -1, dtype=np.int64)
        p[QK_PAD - len(kept):] = kept
        pos.append(p)
    pos = np.stack(pos)  # [B, QK_PAD]

    # qoff[b, ck, jc] = #cols in chunk ck with pos < 128*jc (cols are sorted)
    qoff = np.zeros((B, QKC, NJ + 1), dtype=np.int64)
    for ck in range(QKC):
        pc = pos[:, ck * 512:(ck + 1) * 512]
        for jc in range(NJ + 1):
            qoff[:, ck, jc] = (pc < 128 * jc).sum(axis=1)
    jmax = []   # per chunk: number of key chunks any core needs
    mm_off = []  # baked matmul start col (min over cores)
    mk_end = []  # baked mask end col (max over cores)
    for ck in range(QKC):
        jm = 0
        for jc in range(NJ):
            if (qoff[:, ck, jc] < 512).any():
                jm = jc + 1
        jmax.append(jm)
        mm_off.append([int(qoff[:, ck, jc].min()) for jc in range(NJ)])
        mk_end.append([int(qoff[:, ck, jc + 1].max()) for jc in range(NJ)])
    return pos, qoff, tuple(jmax), mm_off, mk_end


def _build(jmax, mm_off, mk_end, mask_w):
    nc = bacc.Bacc("TRN2", target_bir_lowering=False, debug=False, num_devices=B)
    FP8 = mybir.dt.float8e4
    MW = max(mask_w, 1)
    W0F = mm_off[0][0] & ~3       # pad columns trimmed from xq chunk 0
    QW = [512 - W0F, 512, 512]    # shipped query-chunk widths
    XOFF = EC * 128   # wkv columns at the head of xt
    QOFF = EC * 128   # [wq|wq] columns at the head of xq (duplicated: one
                      # matmul then emits q into both PSUM partition halves)
    # xq layout: [wq | xq0 (trimmed) | xq1 | masks | xq2]
    QB = [QOFF]                     # per-chunk column base inside xq
    QB.append(QOFF + EC * QW[0])
    MSK0 = QB[1] + EC * 512         # masks sit between xq1 and xq2
    QB.append(MSK0 + MW)
    XQW = QB[2] + EC * 512
    xt_ext = nc.dram_tensor("xt", [128, XOFF + KC * EC * 512], BF16, kind="ExternalInput")
    xq_ext = nc.dram_tensor("xq", [128, XQW], FP8, kind="ExternalInput")
    out_ext = nc.dram_tensor("out", [H + 1, QK_PAD], BF16, kind="ExternalOutput")

    with tile.TileContext(nc) as tc:
        with (
            tc.tile_pool(name="const", bufs=1) as const_pool,
            tc.tile_pool(name="acts", bufs=1) as act_pool,
            tc.tile_pool(name="p", bufs=4) as p_pool,
            tc.tile_pool(name="osb", bufs=2) as o_pool,
            tc.tile_pool(name="mmp", bufs=2, space="PSUM") as mmp_pool,
            tc.tile_pool(name="mms", bufs=2, space="PSUM") as mms_pool,
            tc.tile_pool(name="po", bufs=2, space="PSUM") as po_pool,
        ):
            ident = const_pool.tile([128, 128], BF16)
            xt_all = act_pool.tile([128, XOFF + KC * EC * 512], BF16)
            xq_all = act_pool.tile([128, XQW], FP8)
            msk_sb = xq_all[:, MSK0:MSK0 + MW]
            wkv_sb = xt_all[:, 0:XOFF]
            xt_sb = xt_all[:, XOFF:]
            wq_sb = xq_all[:, 0:QOFF]

            # ---- input DMA triggers, in consumer order (sync queue) ----
            half = EC * 512 // 2
            quart = EC * 512 // 4

            def xt_rng(lo, hi):
                nc.sync.dma_start(xt_all[:, lo:hi], xt_ext.ap()[:, lo:hi])

            def xq_rng(lo, hi):
                nc.sync.dma_start(xq_all[:, lo:hi], xq_ext.ap()[:, lo:hi])

            xt_rng(0, XOFF + half)                        # wkv + xt0 e0-3
            xt_rng(XOFF + half, XOFF + EC * 512)          # xt0 e4-7
            xq_rng(0, QB[1])                              # wq + xq0
            xq_rng(QB[1], MSK0)                           # xq1
            xq_rng(MSK0, XQW)                             # masks + xq2
            xt_rng(XOFF + EC * 512, XOFF + 2 * EC * 512)  # xt1
            xt_rng(XOFF + 2 * EC * 512, XOFF + 3 * EC * 512)  # xt2
            xt_rng(XOFF + 3 * EC * 512, XOFF + 3 * EC * 512 + half)  # xt3 e0-3
            xt_rng(XOFF + 3 * EC * 512 + half, XOFF + 4 * EC * 512)  # xt3 e4-7

            make_identity(nc, ident[:])
            # touch Exp once so the ACT table set loads during the DMA phase
            warm = const_pool.tile([1, 1], F32)
            nc.scalar.activation(warm[:], ident[0:1, 0:1],
                                 mybir.ActivationFunctionType.Exp)

            # k and q live in partitions 0:64 only; scores always run on PE
            # array rows 0:64 (serial on the array, but every k-chunk handoff
            # is a single Vector cast — no cross-engine duplication chain,
            # which measured faster than row-tiled score pairs)
            k_sb = act_pool.tile([64, C], BF16)
            vt_sb = act_pool.tile([64, C], BF16)
            q_sb = act_pool.tile([64, QK_PAD], BF16)
            vaug_sb = act_pool.tile([128, NJ * (H + 1)], BF16)
            nc.vector.memset(vaug_sb[:], 1.0)

            I32 = mybir.dt.int32

            def kv_mm(c, pq, e):
                nc.tensor.matmul(
                    pq[:], wkv_sb[:, e * 128:(e + 1) * 128],
                    xt_sb[:, (c * EC + e) * 512:(c * EC + e + 1) * 512],
                    start=(e == 0), stop=(e == EC - 1), skip_group_check=True)

            def kv_casts(c, pq):
                csl = slice(c * 512, (c + 1) * 512)
                nc.vector.tensor_copy(k_sb[:, csl], pq[0:64, :])
                nc.vector.tensor_copy(vt_sb[:, csl], pq[64:128, :])

            def kv_group(c):
                pq = mmp_pool.tile([128, 512], F32, tag="mm", name=f"pq{c}")
                for e in range(EC):
                    kv_mm(c, pq, e)
                kv_casts(c, pq)

            def trp(c):
                for jj in range(4):
                    jc = 4 * c + jj
                    pt = mmp_pool.tile([128, H], BF16, tag="mm", name=f"pt{jc}")
                    nc.tensor.transpose(
                        pt[:], vt_sb[:, jc * 128:(jc + 1) * 128],
                        ident[0:64, 0:64])
                    nc.vector.tensor_copy(
                        vaug_sb[:, jc * (H + 1): jc * (H + 1) + H], pt[:])

            def q_proj(ck):
                # [wq|wq] stationary: one matmul per e-slice emits q into both
                # PSUM partition halves at once; a single [128, w] cast lands
                # q duplicated in both SBUF halves.
                w = QW[ck]
                base = QB[ck]
                pv = mmp_pool.tile([128, 512], F32, tag="mm", name=f"pv{ck}")
                for e in range(EC):
                    q_mm(ck, pv, e)
                q_cast(ck, pv)

            def q_mm(ck, pv, e):
                w = QW[ck]
                base = QB[ck]
                nc.tensor.matmul(
                    pv[:, 0:w], wq_sb[:, e * 128:(e + 1) * 128],
                    xq_all[:, base + e * w:base + (e + 1) * w],
                    start=(e == 0), stop=(e == EC - 1), skip_group_check=True)

            def q_cast(ck, pv):
                w = QW[ck]
                lo = ck * 512 + (512 - w)
                nc.vector.tensor_copy(q_sb[:, lo:(ck + 1) * 512], pv[0:64, 0:w])

            # ---- attention machinery ----
            # mask tile packing offsets (shared layout; content is per-core)
            mask_offs = {}
            off = 0
            for ck in range(QKC):
                for jc in range(jmax[ck]):
                    qo, me = mm_off[ck][jc], mk_end[ck][jc]
                    if me > qo and qo < 512:
                        mask_offs[(ck, jc)] = off
                        off += me - qo

            tiles = {ck: [(jc, mm_off[ck][jc], mk_end[ck][jc])
                          for jc in range(jmax[ck]) if mm_off[ck][jc] < 512]
                     for ck in range(QKC)}
            po_state = {}  # ck -> [po_tile, n_av_done, n_av_total]
            pend = []      # delayed AV work: (ck, pair, p_t)

            def flush_av():
                # AVs run one pair-step late so the PE queue never waits on
                # the exp round-trip; the mask multiply (GpSimd) hides in the
                # same slack.
                while pend:
                    ck, pair, p_t = pend.pop(0)
                    st = po_state[ck]
                    for h, (jc, qo, me) in enumerate(pair):
                        nc.tensor.matmul(
                            st[0][:, qo:512],
                            vaug_sb[:, jc * (H + 1):(jc + 1) * (H + 1)],
                            p_t[:, h * 512 + qo:(h + 1) * 512],
                            start=(st[1] == 0), stop=(st[1] == st[2] - 1))
                        st[1] += 1

            def att_step(ck, pair):
                if ck not in po_state:
                    po_state[ck] = [po_pool.tile([H + 1, 512], F32, tag="po",
                                                 name=f"po{ck}"),
                                    0, len(tiles[ck])]
                ps = mms_pool.tile([128, 1024], F32, tag="mms", name="ps")
                p_t = p_pool.tile([128, 1024], BF16, tag="p", name="p_t")
                for h, (jc, qo, me) in enumerate(pair):
                    nc.tensor.matmul(
                        ps[:, h * 512 + qo:(h + 1) * 512],
                        k_sb[:, jc * 128:(jc + 1) * 128],
                        q_sb[:, ck * 512 + qo:(ck + 1) * 512],
                        start=True, stop=True, skip_group_check=True)
                if len(pair) == 2 and pair[1][1] < 352:
                    lo = pair[0][1]
                    nc.scalar.activation(
                        p_t[:, lo:1024], ps[:, lo:1024],
                        mybir.ActivationFunctionType.Exp, scale=SCALE)
                else:
                    for h, (jc, qo, me) in enumerate(pair):
                        nc.scalar.activation(
                            p_t[:, h * 512 + qo:(h + 1) * 512],
                            ps[:, h * 512 + qo:(h + 1) * 512],
                            mybir.ActivationFunctionType.Exp, scale=SCALE)
                for h, (jc, qo, me) in enumerate(pair):
                    if me > qo:  # boundary mask multiply (host-built content)
                        mo = mask_offs[(ck, jc)]
                        nc.gpsimd.tensor_mul(
                            p_t[:, h * 512 + qo:h * 512 + me],
                            p_t[:, h * 512 + qo:h * 512 + me],
                            msk_sb[:, mo:mo + (me - qo)])
                return (ck, pair, p_t)

            def att(ck, j_lo, j_hi):
                seg = [t for t in tiles[ck] if j_lo <= t[0] <= j_hi]
                i = 0
                while i < len(seg):
                    pair = seg[i:i + 2]
                    item = att_step(ck, pair)
                    # keep TWO steps in flight: scores(i) then AV(i-2), so the
                    # exp(i-1) -> AV(i-1) -> scores(i+1) chain never paces the
                    # PE queue (PSUM slot rotation imposes i-2 anyway)
                    while len(pend) > 1:
                        ck_o, pair_o, p_o = pend.pop(0)
                        st = po_state[ck_o]
                        for h, (jc, qo, me) in enumerate(pair_o):
                            nc.tensor.matmul(
                                st[0][:, qo:512],
                                vaug_sb[:, jc * (H + 1):(jc + 1) * (H + 1)],
                                p_o[:, h * 512 + qo:(h + 1) * 512],
                                start=(st[1] == 0), stop=(st[1] == st[2] - 1))
                            st[1] += 1
                    pend.append(item)
                    i += len(pair)

            def close_part(ck, lo, hi, last):
                # ship unnormalized outT + sums row; the host divides while
                # unsharding (removes the recip chain from the critical tail)
                o_t = o_pool.tile([H + 1, 512], BF16, tag="o", name=f"o{ck}_{lo}")
                if last:
                    nc.scalar.copy(o_t[:, lo:hi], po_state[ck][0][:, lo:hi])
                else:
                    nc.vector.tensor_copy(o_t[:, lo:hi], po_state[ck][0][:, lo:hi])
                nc.sync.dma_start(
                    out_ext.ap()[:, ck * 512 + lo:ck * 512 + hi], o_t[:, lo:hi])

            def att_close(ck, lo=None):
                flush_av()
                st = po_state[ck]
                assert st[1] == st[2], (ck, st[1], st[2])
                close_part(ck, lo if lo is not None else mm_off[ck][0], 512,
                           last=(ck == 2))

            # ---- schedule: consumers emitted as their inputs land, AVs one
            # step late, kv/q/transpose work woven between attention steps ----
            # fused kv0 + q1: q1's matmuls weave into kv0's tail (their xq
            # DMA lands later than xt0's) so q1's cast chases k0's and the
            # exp stream starts as early as the data allows
            pq0 = mmp_pool.tile([128, 512], F32, tag="mm", name="pq0")
            pv1 = mmp_pool.tile([128, 512], F32, tag="mm", name="pv1")
            for e in range(EC):
                kv_mm(0, pq0, e)
                if e >= 4:
                    q_mm(1, pv1, e - 4)
            kv_casts(0, pq0)
            for e in range(4, EC):
                q_mm(1, pv1, e)
            q_cast(1, pv1)
            q_proj(2)
            q_proj(0)
            trp(0)
            att(0, 0, 0)
            att_close(0)
            att(1, 0, 3)
            att(2, 0, 3)
            kv_group(1)
            trp(1)
            att(1, 4, 7)
            att(2, 4, 7)
            kv_group(2)
            trp(2)
            att(1, 8, 8)
            att_close(1)
            kv_group(3)
            trp(3)
            att(2, 8, 11)
            flush_av()
            # columns below the j12 start are final after j11's AV: ship them
            # while the j12-15 scores/exps still run
            qo12 = mm_off[2][12]
            close_part(2, 0, qo12, last=False)
            att(2, 12, 15)
            att_close(2, lo=qo12)

    nc.compile()
    return nc


def _pack_masks(pos, jmax, mm_off, mk_end):
    """Per-core packed boundary masks: msk[j_local, off+q-qo] = (128jc + j_local <= pos[q])."""
    total = 0
    spans = []
    for ck in range(len(jmax)):
        for jc in range(jmax[ck]):
            qo, me = mm_off[ck][jc], mk_end[ck][jc]
            if me > qo and qo < 512:
                spans.append((ck, jc, qo, me, total))
                total += me - qo
    bf = ml_dtypes.bfloat16
    masks = np.zeros((B, 128, max(total, 1)), dtype=np.float32)
    jl = np.arange(128)[:, None]
    for b in range(B):
        for ck, jc, qo, me, off in spans:
            pq = pos[b, ck * 512 + qo: ck * 512 + me][None, :]
            masks[b, :, off:off + (me - qo)] = (128 * jc + jl <= pq)
    return masks.astype(bf), total


def _sbufify(w):  # [E, M] -> [128, EC*M]: w_t[p, e*M+m] = w[e*128+p, m]
    M = w.shape[1]
    return np.ascontiguousarray(
        w.reshape(EC, 128, M).transpose(1, 0, 2).reshape(128, EC * M))


def _retile_cols(xt, ncols, w=512):  # [E, ncols] -> [128, (ncols/w)*EC*w] chunk-major
    return np.ascontiguousarray(
        xt.reshape(EC, 128, ncols // w, w).transpose(1, 2, 0, 3)
        .reshape(128, (ncols // w) * EC * w))


def make_in_maps(x, Wq, Wk, Wv, zero_mask):
    x = np.asarray(x)
    pos, qoff, jmax, mm_off, mk_end = _plan(zero_mask)
    masks, mask_w = _pack_masks(pos, jmax, mm_off, mk_end)
    bf = ml_dtypes.bfloat16
    f8 = ml_dtypes.float8_e4m3fn
    W0F = mm_off[0][0] & ~3
    wkv = _sbufify(np.concatenate([np.asarray(Wk), np.asarray(Wv)], 1)).astype(bf)
    wq = _sbufify(np.concatenate([np.asarray(Wq), np.asarray(Wq)], 1)).astype(f8)
    maps = []
    for b in range(B):
        xtb = np.ascontiguousarray(x[b].T.astype(np.float32))
        xqb = np.zeros((E, QK_PAD), dtype=np.float32)
        real = pos[b] >= 0
        xqb[:, real] = xtb[:, pos[b][real]]
        # chunk 0 ships trimmed (cols W0F:512 only; the rest are pads on
        # every core), chunks 1-2 full width
        xq0 = np.ascontiguousarray(
            xqb[:, W0F:512].reshape(EC, 128, 512 - W0F)
            .transpose(1, 0, 2).reshape(128, EC * (512 - W0F))).astype(f8)
        xq1 = _retile_cols(xqb[:, 512:1024], 512).astype(f8)
        xq2 = _retile_cols(xqb[:, 1024:1536], 512).astype(f8)
        xq_packed = np.concatenate(  # [wq | xq0 | xq1 | masks | xq2]
            [wq, xq0, xq1, masks[b].astype(f8), xq2], axis=1)
        xt_packed = np.concatenate([wkv, _retile_cols(xtb, C).astype(bf)], axis=1)
        maps.append({
            "xt": np.ascontiguousarray(xt_packed),
            "xq": np.ascontiguousarray(xq_packed),
        })
    return maps, (pos, jmax, mm_off, mk_end, mask_w)


def kernel(x, Wq, Wk, Wv, zero_mask):
    in_maps, (pos, jmax, mm_off, mk_end, mask_w) = make_in_maps(
        x, Wq, Wk, Wv, zero_mask)
    key = (jmax, tuple(map(tuple, mm_off)), tuple(map(tuple, mk_end)), mask_w)
    if _CACHED.get("key") != key:
        _CACHED["nc"] = _build(jmax, mm_off, mk_end, mask_w)
        _CACHED["key"] = key
    res = run_bass_kernel_spmd(_CACHED["nc"], in_maps, core_ids=list(range(B)))
    out = np.zeros((B, C, H), dtype=np.float32)
    for b in range(B):
        r = res.results[b]["out"].astype(np.float32)  # [H+1, QK_PAD]
        real = pos[b] >= 0
        out[b][pos[b][real]] = (r[:H, real] / r[H:H + 1, real]).T
    return out
